# revision 1
# baseline (speedup 1.0000x reference)
"""Trainium2 Bass kernel for dual-attention (DisKT-style) nn module.

Math per (batch, head) with S=1024, dk=64, all on-chip in [k, q] layout:
    sT       = (k_h @ q_h^T)            (+ -1e30 on causal-dead diag block)
    E1T      = exp(sT / 8)              (causally-dead region never computed)
    r1[q]    = sum_k E1T[k, q]          (ones^T @ E1T, PSUM broadcast rows)
    p1       = E1T * rec1[q]
    E2''     = exp(p1) - 1              <- bf16-friendly: small values keep
                                           precision; the "+1" of every key
                                           becomes an exact rank-1 vtot fixup
    outT     = (cm*vhi)^T @ E2'' + (cm*vlo)^T @ E2''   (cm pre-masked on host,
                                            v split hi+lo kills bf16 v error)
    r2       = 1024 + cmrep^T @ E2''
    out      = (outT + vtot) * (1/r2) ;  out[:, q=0] = 0
Outputs are produced as [d, q] and transposed back on the host.

Scheduling: the PE must never idle (TRN2 DVFS drops the PE clock from
2.4 GHz to 1.2/0.65 GHz after any idle; full speed needs 3us of
continuous execution).  Scores run through 512-wide single-bank PSUM
windows on a 3-buffer ring so exp1(w) overlaps scores(w+1..w+2); r1
matmuls trail their exp1 by 2 windows; an older block's PV/r2 matmul
quanta are interleaved between windows as PE filler.  r2 is computed
in two sequential 512-col halves on a single psum bank (that frees the
bank for the 3-deep scores ring), with PV-high quanta between the
halves to hide recip2 latency.  qt/kt are zero-padded to 128
contraction rows (64-row matmuls stream ~1.5x slower).  4-deep block
pipeline: A(n) scores/exp1/r1 | M(n-1) p1-muls | X(n-2) exp2+sub |
D(n-3) PV; exp2 pieces are injected mid-way into the Act window
stream so neither exp1 nor exp2 head-blocks the other; chunks >= XCH
use e2 = p1 (expm1(x) ~= x for the small weights at large q), exact
exp2 only where p1 can be large.

Sharding: data-parallel over batch, B=16 -> 2 per core on 8 cores.
"""

import numpy as np
import ml_dtypes

import concourse.bass as bass
import concourse.mybir as mybir
import concourse.tile as tile
from concourse import bacc
from concourse.bass_utils import run_bass_kernel_spmd

B, S, D, H = 16, 1024, 512, 8
DK = D // H           # 64
NCORES = 8
BLOC = B // NCORES    # 2 batches per core
NCH = S // 128        # 8 k-chunks of 128
F32 = mybir.dt.float32
BF16 = mybir.dt.bfloat16
NPBF16 = ml_dtypes.bfloat16

LIVE = [S - 128 * c for c in range(NCH)]          # live width per chunk
OFF = [sum(LIVE[:c]) for c in range(NCH)]         # packed offset per chunk
PACK = OFF[-1] + LIVE[-1]                         # 4608
NW = PACK // 512                                  # 9 scores windows of 512
# window w covers packed cols [512w, 512w+512): list of (chunk, a, b) with
# a/b chunk-live-local
WPIECES = []
for _w in range(NW):
    _s0, _s1 = 512 * _w, 512 * (_w + 1)
    _ps = []
    for _c in range(NCH):
        _a, _b = max(_s0, OFF[_c]), min(_s1, OFF[_c] + LIVE[_c])
        if _a < _b:
            _ps.append((_c, _a - OFF[_c], _b - OFF[_c]))
    WPIECES.append(_ps)

# number of v parts in the PV sweep: 2 = hi+lo (accurate), 1 = hi only (fast)
VSPLIT = 1
# chunks < XCH get exact exp2 (expm1(x)~=x elsewhere; exact where p1 large)
XCH = 1
# chunks whose mul runs f32 on DVE then converts to e2 via tensor_copy
CONV = set()
# chunks whose (direct bf16) mul runs on DVE rather than GpSimd
MUL_ON_DVE = {1}
# exp2/sub splits (packed col ranges, within the exact region) and the
# window after which each is emitted into the Act stream
XSPLITS = [(0, 512), (512, 1024)]
XPOS = {3: 0, 6: 1}
# D-quanta drained before w0 and after each window
PRE_DRAIN = 2
DRAINS = [2, 2, 2, 2, 2, 2, 2, 2, 2]

# knobs that test.py can flip
TRACE = False
LAST_RESULTS = None


def build_nc(debug=False):
    nc = bacc.Bacc("TRN2", target_bir_lowering=False, debug=debug)
    AF = mybir.ActivationFunctionType
    ALU = mybir.AluOpType

    qt_d = nc.dram_tensor("qt", [BLOC, H, 128, S], BF16, kind="ExternalInput")
    kt_d = nc.dram_tensor("kt", [BLOC, H, 128, S], BF16, kind="ExternalInput")
    # (1-cm)*[v1|v2] per (b, h, chunk), split into bf16 hi + lo parts
    vcat_d = nc.dram_tensor(
        "vcat", [BLOC, H, NCH, 2, 128, 128], BF16, kind="ExternalInput"
    )
    vtot_d = nc.dram_tensor("vtot", [BLOC, H, 128], F32, kind="ExternalInput")
    # (1-cm) replicated across columns, per (b, chunk): r2 matmul weights
    cmrep_d = nc.dram_tensor("cmrep", [BLOC, NCH, 128, 128], BF16, kind="ExternalInput")
    ind_d = nc.dram_tensor("ind", [128, S], BF16, kind="ExternalInput")
    cnt_d = nc.dram_tensor("cnt", [128, 128], BF16, kind="ExternalInput")
    dmask_d = nc.dram_tensor("dmask", [128, 128], BF16, kind="ExternalInput")
    ident_d = nc.dram_tensor("ident", [128, 128], BF16, kind="ExternalInput")
    ones_d = nc.dram_tensor("onesd", [128, 128], BF16, kind="ExternalInput")
    out1_d = nc.dram_tensor("out1t", [BLOC, D, S], F32, kind="ExternalOutput")
    out2_d = nc.dram_tensor("out2t", [BLOC, D, S], F32, kind="ExternalOutput")

    def bank_pieces(p0, p1):
        """split [p0, p1) at 512-aligned psum bank boundaries"""
        out = []
        p = p0
        while p < p1:
            end = min(p1, (p // 512 + 1) * 512)
            out.append((p, end))
            p = end
        return out

    with tile.TileContext(nc) as tc:
        with (
            tc.tile_pool(name="consts", bufs=1) as consts,
            tc.tile_pool(name="qk", bufs=3) as qkp,
            tc.tile_pool(name="vc", bufs=5) as vcp,
            tc.tile_pool(name="e1", bufs=3) as e1p,
            tc.tile_pool(name="e2", bufs=3) as e2p,
            tc.tile_pool(name="tmp", bufs=2) as tmpp,
            tc.tile_pool(name="rc", bufs=3) as rcp,
            tc.tile_pool(name="outs", bufs=2) as outp,
            tc.tile_pool(name="sc_ps", bufs=3, space="PSUM") as sc_psp,
            tc.tile_pool(name="r_ps", bufs=1, space="PSUM") as r_psp,
            tc.tile_pool(name="o_ps", bufs=1, space="PSUM") as o_psp,
        ):
            dm_sb = consts.tile([128, 128], BF16)
            nc.sync.dma_start(out=dm_sb, in_=dmask_d[:, :])
            id_sb = consts.tile([128, 128], BF16)
            nc.sync.dma_start(out=id_sb, in_=ident_d[:, :])
            ones_sb = consts.tile([128, 128], BF16)
            nc.sync.dma_start(out=ones_sb, in_=ones_d[:, :])

            NB = BLOC * H
            st = [dict() for _ in range(NB)]

            def late_consts():
                nonlocal vtot_sb, ind_sb, cnt_sb, cmrep_sb
                vtot_sb = consts.tile([128, BLOC * H], F32, name="vtot_sb")
                nc.sync.dma_start(
                    out=vtot_sb, in_=vtot_d[:].rearrange("b h d -> d (b h)")
                )
                ind_sb = consts.tile([128, S], BF16, name="ind_sb")
                nc.sync.dma_start(out=ind_sb, in_=ind_d[:, :])
                cnt_sb = consts.tile([128, 128], BF16, name="cnt_sb")
                nc.sync.dma_start(out=cnt_sb, in_=cnt_d[:, :])
                cmrep_sb = consts.tile([128, BLOC, NCH, 128], BF16, name="cmrep_sb")
                nc.sync.dma_start(
                    out=cmrep_sb, in_=cmrep_d[:].rearrange("b c p j -> p b c j")
                )

            vtot_sb = ind_sb = cnt_sb = cmrep_sb = None

            # PE warmup: dependency-free matmuls on a memset scratch tile
            # overlap the initial DMAs and pre-ramp the PE clock (DVFS)
            wu_sb = consts.tile([128, 512], BF16, name="wu_sb")
            nc.vector.memset(wu_sb, 0.0)
            wu_ps = sc_psp.tile([128, 512], F32, tag="sc", name="wu_ps")
            for _ in range(12):
                nc.tensor.matmul(
                    wu_ps,
                    lhsT=wu_sb[:, 0:128],
                    rhs=wu_sb,
                    start=True,
                    stop=True,
                    skip_group_check=True,
                )

            def dma_in(blk):
                bi, h = divmod(blk, H)
                s = st[blk]
                qt_sb = qkp.tile([128, S], BF16, tag="qt")
                kt_sb = qkp.tile([128, S], BF16, tag="kt")
                nc.sync.dma_start(out=qt_sb, in_=qt_d[bi, h])
                nc.sync.dma_start(out=kt_sb, in_=kt_d[bi, h])
                vc_sb = vcp.tile([128, NCH, VSPLIT, 128], BF16, tag="vc")
                nc.sync.dma_start(
                    out=vc_sb,
                    in_=vcat_d[bi, h, :, 0:VSPLIT].rearrange("c l p j -> p c l j"),
                )
                s["qt"], s["kt"], s["vc"] = qt_sb, kt_sb, vc_sb

            def emit_sc(blk, w):
                """scores matmuls for packed window w into a 1-bank psum tile"""
                s = st[blk]
                sps = sc_psp.tile([128, 512], F32, tag="sc", name="sps")
                s["sps"][w] = sps
                base = 512 * w
                for c, a, b in WPIECES[w]:
                    q0 = 128 * c
                    loc = OFF[c] + a - base
                    nc.tensor.matmul(
                        sps[:, loc : loc + (b - a)],
                        lhsT=s["kt"][:, q0 : q0 + 128],
                        rhs=s["qt"][:, q0 + a : q0 + b],
                        start=True,
                        stop=(a > 0),
                        skip_group_check=True,
                    )
                    if a == 0:
                        # causal: += I^T @ dmask adds -1e30 on/above diag
                        nc.tensor.matmul(
                            sps[:, loc : loc + 128],
                            lhsT=id_sb,
                            rhs=dm_sb,
                            start=False,
                            stop=True,
                            skip_group_check=True,
                        )

            def emit_exp1(blk, w):
                s = st[blk]
                if w == 0:
                    s["e1"] = e1p.tile([128, PACK], BF16, tag="e1", name="e1")
                base = 512 * w
                nc.scalar.activation(
                    s["e1"][:, base : base + 512],
                    s["sps"][w][:, 0:512],
                    AF.Exp,
                    scale=0.125,
                )

            def emit_r1(blk, w):
                s = st[blk]
                if w == 0:
                    s["r1ps"] = r_psp.tile([128, S], F32, tag="r1", name="r1ps")
                for c, a, b in WPIECES[w]:
                    q0 = 128 * c
                    for p0, p1 in bank_pieces(q0 + a, q0 + b):
                        nc.tensor.matmul(
                            s["r1ps"][:, p0:p1],
                            lhsT=ones_sb,
                            rhs=s["e1"][:, OFF[c] + p0 - q0 : OFF[c] + p1 - q0],
                            start=(c == 0),
                            stop=(c == NCH - 1),
                            skip_group_check=True,
                        )

            def emit_recip1(blk):
                s = st[blk]
                rec1 = rcp.tile([128, S], F32, tag="rec1")
                nc.vector.reciprocal_approx_fast(out=rec1, in_=s["r1ps"][:, 0:S])
                nc.vector.memset(rec1[:, 0:1], 0.0)
                s["rec1"] = rec1

            def phase_M(blk):
                """p1 = e1 * rec1: chunks < XCH -> tmp f32 (exact exp2
                follows); CONV chunks -> tmp f32 then copy to e2 (bf16-out
                tensor_tensor is slow on DVE); others -> e2 on GpSimd"""
                s = st[blk]
                tmax = max([XCH - 1] + list(CONV))
                tmp = tmpp.tile([128, OFF[tmax] + LIVE[tmax]], F32, tag="tmp")
                s["tmp"] = tmp
                s["e2"] = e2p.tile([128, PACK], BF16, tag="e2", name="e2")
                for c in range(NCH):
                    q0 = 128 * c
                    sl = slice(OFF[c], OFF[c] + LIVE[c])
                    if c < XCH or c in CONV:
                        nc.vector.tensor_mul(
                            tmp[:, sl], s["e1"][:, sl], s["rec1"][:, q0:S]
                        )
                        if c in CONV:
                            nc.vector.tensor_copy(out=s["e2"][:, sl], in_=tmp[:, sl])
                    else:
                        eng = nc.vector if c in MUL_ON_DVE else nc.gpsimd
                        eng.tensor_mul(
                            s["e2"][:, sl], s["e1"][:, sl], s["rec1"][:, q0:S]
                        )

            def emit_X(blk, piece):
                """exact exp2 + sub for one chunk-0 split -> e2 bf16"""
                s = st[blk]
                x0, x1 = XSPLITS[piece]
                nc.scalar.activation(s["tmp"][:, x0:x1], s["tmp"][:, x0:x1], AF.Exp)
                nc.vector.tensor_scalar_add(
                    s["e2"][:, x0:x1], s["tmp"][:, x0:x1], -1.0
                )

            def d_quanta(blk):
                """PE-filler quanta for the PV/r2 sweep of an older block.
                r2 runs in two sequential 512-col halves on a single psum
                bank (frees a bank for the 3-deep scores ring); PV-high
                quanta sit between the halves to hide recip2-A latency."""
                bi, h = divmod(blk, H)
                s = st[blk]
                qs = []

                def q_ot_alloc():
                    s["otps"] = o_psp.tile([128, S], F32, tag="ot", name="otps")

                qs.append(q_ot_alloc)
                low, high = [], []
                for c in range(NCH):
                    q0 = 128 * c
                    for p0, p1 in bank_pieces(q0, S):
                        (low if p0 < 512 else high).append((c, q0, p0, p1))

                def mk_pv(c, q0, p0, p1):
                    def q_pv():
                        ee = s["e2"][:, OFF[c] + p0 - q0 : OFF[c] + p1 - q0]
                        for hl in range(VSPLIT):
                            nc.tensor.matmul(
                                s["otps"][:, p0:p1],
                                lhsT=s["vc"][:, c, hl, :],
                                rhs=ee,
                                start=(c == 0 and hl == 0),
                                stop=(c == NCH - 1 and hl == VSPLIT - 1),
                                skip_group_check=True,
                            )

                    return q_pv

                for c, q0, p0, p1 in low:
                    qs.append(mk_pv(c, q0, p0, p1))

                def q_r2a1():
                    s["r2psA"] = r_psp.tile([128, 512], F32, tag="r2", name="r2psA")
                    nc.tensor.matmul(
                        s["r2psA"][:, 0:512],
                        lhsT=cnt_sb,
                        rhs=ind_sb[:, 0:512],
                        start=True,
                        stop=False,
                        skip_group_check=True,
                    )
                    nc.tensor.matmul(
                        s["r2psA"][:, 0:512],
                        lhsT=cmrep_sb[:, bi, 0, :],
                        rhs=s["e2"][:, 0:512],
                        start=False,
                        stop=False,
                        skip_group_check=True,
                    )

                def q_r2a2():
                    for c in range(1, 4):
                        q0 = 128 * c
                        nc.tensor.matmul(
                            s["r2psA"][:, q0:512],
                            lhsT=cmrep_sb[:, bi, c, :],
                            rhs=s["e2"][:, OFF[c] : OFF[c] + 512 - q0],
                            start=False,
                            stop=(c == 3),
                            skip_group_check=True,
                        )

                qs += [q_r2a1, q_r2a2]

                def q_finA():
                    rec2a = rcp.tile([128, 512], F32, tag="rec2", name="rec2a")
                    nc.vector.reciprocal_approx_fast(
                        out=rec2a, in_=s["r2psA"][:, 0:512]
                    )
                    ot_sb = outp.tile([128, S], F32, tag="otsb", name="ot_sb")
                    s["ot_sb"] = ot_sb
                    nc.vector.scalar_tensor_tensor(
                        out=ot_sb[:, 0:512],
                        in0=s["otps"][:, 0:512],
                        scalar=vtot_sb[:, blk : blk + 1],
                        in1=rec2a,
                        op0=ALU.add,
                        op1=ALU.mult,
                    )
                    nc.vector.memset(ot_sb[:, 0:1], 0.0)
                    nc.sync.dma_start(
                        out=out1_d[bi, DK * h : DK * (h + 1), 0:512],
                        in_=ot_sb[0:DK, 0:512],
                    )
                    nc.sync.dma_start(
                        out=out2_d[bi, DK * h : DK * (h + 1), 0:512],
                        in_=ot_sb[DK : 2 * DK, 0:512],
                    )

                qs.append(q_finA)
                for c, q0, p0, p1 in high:
                    qs.append(mk_pv(c, q0, p0, p1))

                def q_r2b1():
                    s["r2psB"] = r_psp.tile([128, 512], F32, tag="r2", name="r2psB")
                    nc.tensor.matmul(
                        s["r2psB"][:, 0:512],
                        lhsT=cnt_sb,
                        rhs=ind_sb[:, 512:1024],
                        start=True,
                        stop=False,
                        skip_group_check=True,
                    )
                    for c in range(0, 2):
                        q0 = 128 * c
                        nc.tensor.matmul(
                            s["r2psB"][:, 0:512],
                            lhsT=cmrep_sb[:, bi, c, :],
                            rhs=s["e2"][:, OFF[c] + 512 - q0 : OFF[c] + 1024 - q0],
                            start=False,
                            stop=False,
                            skip_group_check=True,
                        )

                def q_r2b2():
                    for c in range(2, 4):
                        q0 = 128 * c
                        nc.tensor.matmul(
                            s["r2psB"][:, 0:512],
                            lhsT=cmrep_sb[:, bi, c, :],
                            rhs=s["e2"][:, OFF[c] + 512 - q0 : OFF[c] + 1024 - q0],
                            start=False,
                            stop=False,
                            skip_group_check=True,
                        )

                def q_r2b3():
                    for c in range(4, NCH):
                        q0 = 128 * c
                        nc.tensor.matmul(
                            s["r2psB"][:, q0 - 512 : 512],
                            lhsT=cmrep_sb[:, bi, c, :],
                            rhs=s["e2"][:, OFF[c] : OFF[c] + 1024 - q0],
                            start=False,
                            stop=(c == NCH - 1),
                            skip_group_check=True,
                        )

                qs += [q_r2b1, q_r2b2, q_r2b3]

                def q_finB():
                    rec2b = rcp.tile([128, 512], F32, tag="rec2", name="rec2b")
                    nc.vector.reciprocal_approx_fast(
                        out=rec2b, in_=s["r2psB"][:, 0:512]
                    )
                    ot_sb = s["ot_sb"]
                    nc.vector.scalar_tensor_tensor(
                        out=ot_sb[:, 512:1024],
                        in0=s["otps"][:, 512:1024],
                        scalar=vtot_sb[:, blk : blk + 1],
                        in1=rec2b,
                        op0=ALU.add,
                        op1=ALU.mult,
                    )
                    nc.sync.dma_start(
                        out=out1_d[bi, DK * h : DK * (h + 1), 512:1024],
                        in_=ot_sb[0:DK, 512:1024],
                    )
                    nc.sync.dma_start(
                        out=out2_d[bi, DK * h : DK * (h + 1), 512:1024],
                        in_=ot_sb[DK : 2 * DK, 512:1024],
                    )

                qs.append(q_finB)
                return qs

            # 4-deep pipeline: A(n) | M(n-1) | X(n-2) | D(n-3), with D's
            # matmul quanta interleaved into A's window stream as PE filler
            dma_in(0)
            late_consts()
            for i in range(NB + 3):
                dq = d_quanta(i - 3) if 3 <= i < NB + 3 else []
                di = 0

                def drain(k):
                    nonlocal di
                    n = min(k, len(dq) - di)
                    for _ in range(n):
                        dq[di]()
                        di += 1

                if i < NB:
                    if i + 1 < NB:
                        dma_in(i + 1)
                    st[i]["sps"] = {}
                    drain(PRE_DRAIN)
                    for w in range(NW):
                        emit_sc(i, w)
                        emit_exp1(i, w)
                        if i >= 2 and w in XPOS:
                            emit_X(i - 2, XPOS[w])
                        drain(DRAINS[w])
                        if w >= 3:
                            emit_r1(i, w - 3)
                    emit_r1(i, NW - 3)
                    emit_r1(i, NW - 2)
                    emit_r1(i, NW - 1)
                    drain(len(dq))
                    emit_recip1(i)
                else:
                    if i - 2 < NB:
                        for p in range(len(XSPLITS)):
                            emit_X(i - 2, p)
                    drain(len(dq))
                if 1 <= i <= NB:
                    phase_M(i - 1)

    nc.compile()
    return nc


_NC_CACHE = None


def _get_nc():
    global _NC_CACHE
    if _NC_CACHE is None:
        _NC_CACHE = build_nc()
    return _NC_CACHE


def make_in_maps(q, k, v1, v2, cm):
    """Full inputs -> per-core input maps (host-side sharding + layout)."""
    q = np.asarray(q, dtype=np.float32).astype(NPBF16)
    k = np.asarray(k, dtype=np.float32).astype(NPBF16)
    v1 = np.asarray(v1, dtype=np.float32)
    v2 = np.asarray(v2, dtype=np.float32)
    cm = np.asarray(cm)

    # additive causal mask for the diagonal block: 0 where k < q else -1e30
    dmask = np.where(
        np.arange(128)[:, None] < np.arange(128)[None, :], 0.0, -1e30
    ).astype(NPBF16)
    ident = np.eye(128, dtype=NPBF16)
    onesd = np.ones((128, 128), NPBF16)
    ind = np.ones((128, S), np.float32).astype(NPBF16)
    cnt = np.full((128, 128), float(S) / 128.0, np.float32).astype(NPBF16)

    in_maps = []
    for core in range(NCORES):
        b0 = core * BLOC
        qt = np.zeros((BLOC, H, 128, S), NPBF16)  # [b, h, dk(pad 128), s]
        qt[:, :, 0:DK] = q[b0 : b0 + BLOC].reshape(BLOC, S, H, DK).transpose(0, 2, 3, 1)
        kt = np.zeros((BLOC, H, 128, S), NPBF16)
        kt[:, :, 0:DK] = k[b0 : b0 + BLOC].reshape(BLOC, S, H, DK).transpose(0, 2, 3, 1)
        cml = 1.0 - cm[b0 : b0 + BLOC].astype(np.float32)  # [b, s] (1-cm)
        v1s = v1[b0 : b0 + BLOC].reshape(BLOC, NCH, 128, H, DK).transpose(0, 3, 1, 2, 4)
        v2s = v2[b0 : b0 + BLOC].reshape(BLOC, NCH, 128, H, DK).transpose(0, 3, 1, 2, 4)
        vc = np.empty((BLOC, H, NCH, 128, 128), np.float32)
        vc[..., 0:DK] = v1s
        vc[..., DK : 2 * DK] = v2s
        # vtot: unmasked total column sums (the "+1" of every key)
        vtot = np.ascontiguousarray(
            vc.astype(np.float64).sum(axis=(2, 3)).astype(np.float32)
        )  # [b,h,128]
        # counter-mask folded into the PV weights
        vcm = vc * cml.reshape(BLOC, 1, NCH, 128, 1)
        vhi = vcm.astype(NPBF16)
        vlo = (vcm - vhi.astype(np.float32)).astype(NPBF16)
        vcat = np.ascontiguousarray(np.stack([vhi, vlo], axis=3))
        cmrep = np.ascontiguousarray(
            np.broadcast_to(
                cml.reshape(BLOC, NCH, 128, 1), (BLOC, NCH, 128, 128)
            ).astype(NPBF16)
        )
        in_maps.append(
            dict(
                qt=qt, kt=kt, vcat=vcat, vtot=vtot, cmrep=cmrep,
                ind=ind, cnt=cnt, dmask=dmask, ident=ident, onesd=onesd,
            )
        )
    return in_maps


def _gather(res):
    out1 = np.concatenate(
        [r["out1t"].transpose(0, 2, 1) for r in res.results], axis=0
    )
    out2 = np.concatenate(
        [r["out2t"].transpose(0, 2, 1) for r in res.results], axis=0
    )
    return np.ascontiguousarray(out1), np.ascontiguousarray(out2)


def kernel(q, k, v1, v2, counter_attention_mask):
    global LAST_RESULTS
    in_maps = make_in_maps(q, k, v1, v2, counter_attention_mask)
    nc = _get_nc()
    res = run_bass_kernel_spmd(
        nc, in_maps, core_ids=list(range(NCORES)), trace=TRACE
    )
    LAST_RESULTS = res
    return _gather(res)



# revision 16
# speedup vs baseline: 1.0758x; 1.0758x over previous
"""Trainium2 Bass kernel for dual-attention (DisKT-style) nn module.

Math per (batch, head), S=1024, dk=64, [k, q] layout on-chip:
    sT   = (k_h @ q_h^T) + causal(-448 fp8 fixup)       fp8 matmuls
    e1   = fp8(exp(sT/8))                               ACT, fp8 out
    r1s  = 256 * sum_k e1[k, q]                         fp8 pair-DoubleRow
    otps = (vcm/4)^T @ e1                               fp8 pair-DoubleRow
    out[q>=128] = otps * recip(r1s) + vtot/1024
    exact island q<128 (second-softmax exp matters there):
      p1m  = e1 * (256*cml[k]) * recip(r1s[0:128])
      e2x  = exp(p1m)   (==1 at masked/dead keys)
      out[0:128] = ((vcm0/4)^T @ e2x + vtot0adj/4) / 256

Key simplifications vs a literal translation of the reference:
  - second softmax denominator = 1024 + sum cml*(exp(p1)-1) is in
    [1024, 1025] since sum p1 <= 1: approximated by 1024 (<=1e-3 rel).
  - the "+1 per key" of the second softmax contributes vtot[d]*rec2 ~=
    vtot/1024: folded into a per-partition scalar add (exact vtot).
  - exp(p1) ~= 1 + p1 for q >= 128 (p1 <= ~0.1): the PV/r1 matmuls run
    directly on e1 and the 1/r1 scale folds into the output pass.
  - chunk-pair DoubleRow fp8: moving AP [128, (delta,2), (1,W)] feeds
    two 128-key chunks per pass instruction at 0.5 cyc/col.

Sharding: data-parallel over batch, B=16 -> 2 per core on 8 cores.
"""

import numpy as np
import ml_dtypes

import concourse.bass as bass
import concourse.mybir as mybir
import concourse.tile as tile
from concourse import bacc
from concourse.ap import AP
from concourse.bass_utils import run_bass_kernel_spmd

B, S, D, H = 16, 1024, 512, 8
DK = D // H           # 64
NCORES = 8
BLOC = B // NCORES    # 2 batches per core
NB = BLOC * H         # 16 blocks per core
NCH = S // 128        # 8 k-chunks of 128
F32 = mybir.dt.float32
BF16 = mybir.dt.bfloat16
F8 = mybir.dt.float8e4
DR = mybir.MatmulPerfMode.DoubleRow
NPBF16 = ml_dtypes.bfloat16
NPF8 = ml_dtypes.float8_e4m3

LIVE = [S - 128 * c for c in range(NCH)]          # live width per chunk
OFF = [sum(LIVE[:c]) for c in range(NCH)]         # packed offset per chunk
PACK = OFF[-1] + LIVE[-1]                         # 4608
E1W = PACK + 128                                  # +128 zero cols for A-only
NW = (PACK + 1023) // 1024                        # 5 exp windows
WLEN = [min(1024, PACK - 1024 * w) for w in range(NW)]

# knobs test.py can flip
TRACE = False
DEBUG_DUMP = False
DBG_BLK = 0
LAST_RESULTS = None


def _scores_segs():
    """[(w, s0, s1, c, d0, d1)]: packed-col segments per exp window, split
    at 512 psum-bank boundaries; (d0, d1) = local diag-fixup range or None"""
    segs = []
    for c in range(NCH):
        g0, g1 = OFF[c], OFF[c] + LIVE[c]
        bounds = sorted({g0, g1} | {x for x in range(0, PACK + 512, 512)
                                    if g0 < x < g1})
        for s0, s1 in zip(bounds[:-1], bounds[1:]):
            w = s0 // 1024
            dm = None
            if s0 < g0 + 128:
                dm = (s0 - g0, min(s1 - g0, 128))
            segs.append((w, s0, s1, c, dm))
    return segs


SCORE_SEGS = _scores_segs()


def _pair_pieces(include_cell0):
    """[(pair, a, b, start, stop, aonly)] for a pair-DoubleRow sweep over e1.
    Cells of 256 q-cols; pair p = chunks (2p, 2p+1) covers q >= 256p; the
    first 128 cols of a pair's own cell are A-only (B reads the zero pad).
    CELL-major: each cell's accumulation group opens and closes before the
    next cell starts -- the psum hardware allows only ONE open group per
    bank, so groups must never interleave within a bank."""
    out = []
    for cell in range(4):
        mem = []
        for p in range(cell):
            mem.append([p, 256 * cell, 256 * cell + 256, False, False, False])
        if include_cell0 or cell > 0:
            mem.append([cell, 256 * cell, 256 * cell + 128, False, True, True])
        mem.append([cell, 256 * cell + 128, 256 * cell + 256, False, True, False])
        if cell == 0:
            for m in mem:
                m[3] = True
        else:
            mem[0][3] = True
        out += [(m[0], m[1], m[2], m[3], m[4], m[5]) for m in mem]
    return out


R1_PIECES = _pair_pieces(True)     # 15 pieces, covers q in [0, 1024)
PV_PIECES = _pair_pieces(False)    # 14 pieces, covers q in [128, 1024)


def build_nc(debug=False):
    nc = bacc.Bacc("TRN2", target_bir_lowering=False, debug=debug)
    AF = mybir.ActivationFunctionType
    ALU = mybir.AluOpType

    qt_d = nc.dram_tensor("qt", [BLOC, H, DK, S], F8, kind="ExternalInput")
    kt_d = nc.dram_tensor("kt", [BLOC, H, DK, S], F8, kind="ExternalInput")
    # pair-DR PV weights: vcm/4 as [pair, key, slot, d]
    vcp_d = nc.dram_tensor("vcp", [BLOC, H, 128, 4, 2, 128], F8,
                           kind="ExternalInput")
    # exact-path chunk-0 weights (cml*v)/4
    vcm0_d = nc.dram_tensor("vcm0", [BLOC, H, 128, 128], BF16,
                            kind="ExternalInput")
    vt1024_d = nc.dram_tensor("vt1024", [128, NB], F32, kind="ExternalInput")
    vt0adj_d = nc.dram_tensor("vt0adj", [128, NB], F32, kind="ExternalInput")
    cml256_d = nc.dram_tensor("cml256", [128, BLOC], F32, kind="ExternalInput")
    ones256_d = nc.dram_tensor("ones256", [128, 2, 128], F8, kind="ExternalInput")
    id8_d = nc.dram_tensor("id8", [128, 128], F8, kind="ExternalInput")
    dm8_d = nc.dram_tensor("dm8", [128, 128], F8, kind="ExternalInput")
    out1_d = nc.dram_tensor("out1t", [BLOC, D, S], BF16, kind="ExternalOutput")
    out2_d = nc.dram_tensor("out2t", [BLOC, D, S], BF16, kind="ExternalOutput")
    if DEBUG_DUMP:
        e1dump_d = nc.dram_tensor("e1dump", [128, E1W], F8, kind="ExternalOutput")
        r1dump_d = nc.dram_tensor("r1dump", [128, 1024], F32, kind="ExternalOutput")
        otdump_d = nc.dram_tensor("otdump", [128, 1024], F32, kind="ExternalOutput")

    with tile.TileContext(nc) as tc:
        with (
            tc.tile_pool(name="consts", bufs=1) as consts,
            tc.tile_pool(name="vc", bufs=2) as vcp_p,
            tc.tile_pool(name="xs", bufs=2) as xsp,
            tc.tile_pool(name="fin", bufs=2) as finp,
            tc.tile_pool(name="outs", bufs=2) as outp,
            tc.tile_pool(name="ps", bufs=1, space="PSUM") as psp,
        ):
            id8_sb = consts.tile([128, 128], F8)
            nc.sync.dma_start(out=id8_sb, in_=id8_d[:, :])
            dm8_sb = consts.tile([128, 128], F8)
            nc.sync.dma_start(out=dm8_sb, in_=dm8_d[:, :])
            ones256_sb = consts.tile([128, 2, 128], F8)
            nc.sync.dma_start(out=ones256_sb, in_=ones256_d[:])
            vt1024_sb = consts.tile([128, NB], F32)
            nc.sync.dma_start(out=vt1024_sb, in_=vt1024_d[:, :])
            vt0adj_sb = consts.tile([128, NB], F32)
            nc.sync.dma_start(out=vt0adj_sb, in_=vt0adj_d[:, :])
            cml256_sb = consts.tile([128, BLOC], F32)
            nc.sync.dma_start(out=cml256_sb, in_=cml256_d[:, :])
            negtwo_sb = consts.tile([128, 1], F32, name="negtwo")
            nc.vector.memset(negtwo_sb, -3.5)

            # persistent 3-deep rings for qt/kt/e1 (stable identity so the
            # one-time pad memsets cover all blocks)
            qt_t = [consts.tile([128, S], F8, name=f"qtr{r}") for r in range(3)]
            kt_t = [consts.tile([128, S], F8, name=f"ktr{r}") for r in range(3)]
            e1_t = [consts.tile([128, E1W], F8, name=f"e1r{r}") for r in range(3)]
            for r in range(3):
                nc.vector.memset(qt_t[r][DK:128, :], 0.0)
                nc.vector.memset(kt_t[r][DK:128, :], 0.0)
                nc.vector.memset(e1_t[r][:, PACK:E1W], 0.0)

            # persistent psum: scores ring (2x 1024), otps, r1s
            ring = psp.tile([128, 2048], F32, name="ring")
            otps = psp.tile([128, 1024], F32, name="otps")
            r1s = psp.tile([128, 1024], F32, name="r1s")

            st = [dict() for _ in range(NB)]

            # PE warmup (DVFS pre-ramp) on a zeroed scratch
            wu_sb = consts.tile([128, 512], BF16, name="wu_sb")
            nc.vector.memset(wu_sb, 0.0)
            for r in range(12):
                nc.tensor.matmul(
                    ring[:, 0:512], lhsT=wu_sb[:, 0:128], rhs=wu_sb,
                    start=True, stop=True, skip_group_check=True,
                )

            def dma_in(blk):
                bi, h = divmod(blk, H)
                s = st[blk]
                qt_sb = qt_t[blk % 3]
                kt_sb = kt_t[blk % 3]
                nc.sync.dma_start(out=qt_sb[0:DK, :], in_=qt_d[bi, h])
                nc.sync.dma_start(out=kt_sb[0:DK, :], in_=kt_d[bi, h])
                vcp_sb = vcp_p.tile([128, 4, 2, 128], F8, tag="vcp")
                nc.sync.dma_start(out=vcp_sb, in_=vcp_d[bi, h])
                vcm0_sb = vcp_p.tile([128, 128], BF16, tag="vcm0")
                nc.sync.dma_start(out=vcm0_sb, in_=vcm0_d[bi, h])
                s["qt"], s["kt"], s["vcp"], s["vcm0"] = qt_sb, kt_sb, vcp_sb, vcm0_sb

            def alloc_e1(blk):
                st[blk]["e1"] = e1_t[blk % 3]

            def emit_scores(blk, w):
                s = st[blk]
                base = (w % 2) * 1024
                g0 = 1024 * w
                for (sw, s0, s1, c, dm) in SCORE_SEGS:
                    if sw != w:
                        continue
                    loc = base + s0 - g0
                    qa = s0 - OFF[c] + 128 * c
                    nc.tensor.matmul(
                        ring[:, loc:loc + (s1 - s0)],
                        lhsT=s["kt"][:, 128 * c:128 * c + 128],
                        rhs=s["qt"][:, qa:qa + (s1 - s0)],
                        start=True, stop=dm is None, skip_group_check=True,
                    )
                    if dm is not None:
                        d0, d1 = dm
                        nc.tensor.matmul(
                            ring[:, loc:loc + (d1 - d0)],
                            lhsT=id8_sb, rhs=dm8_sb[:, d0:d1],
                            start=False, stop=True, skip_group_check=True,
                        )

            def emit_exp1(blk, w):
                s = st[blk]
                base = (w % 2) * 1024
                # bias -3.5: e1 scaled by e^-3.5 so fp8 max 240 is never hit
                # (global max score ~67.8 -> e1max ~144)
                # (every consumer is scale-invariant in e1)
                nc.scalar.activation(
                    s["e1"][:, 1024 * w:1024 * w + WLEN[w]],
                    ring[:, base:base + WLEN[w]],
                    AF.Exp, scale=0.125, bias=negtwo_sb[:, 0:1],
                )

            def pair_mm(s, psum, pieces_sel, lhsT_of):
                """emit pair-DR matmuls for the given piece list"""
                e1t = s["e1"]
                for (p, a, b, st_, sp_, aonly) in pieces_sel:
                    a0 = OFF[2 * p] + (a - 256 * p)
                    delta = (PACK - a0) if aonly else (LIVE[2 * p] - 128)
                    rhs = AP(e1t[:, 0:1].tensor, a0,
                             [[E1W, 128], [delta, 2], [1, b - a]])
                    nc.tensor.matmul(
                        psum[:, a:b], lhsT=lhsT_of(p), rhs=rhs,
                        start=st_, stop=sp_, perf_mode=DR,
                        skip_group_check=True,
                    )

            def quanta(blk):
                """PE/DVE/DMA quanta for block blk's passes + finalize, plus
                the exact-path tail of block blk-1.  Ordering rule: a psum
                READ (DVE) is never emitted right after its producing matmul
                -- several quanta of unrelated PE work sit in between so the
                PE psum-write pipeline has drained by the time the read's
                semaphore fires (observed transient-garbage reads on HW
                otherwise)."""
                bi, h = divmod(blk, H)
                s = st[blk]
                qs = []

                # r1 pass piece for cell 0 first (unlocks exact path)
                qs.append(lambda: pair_mm(s, r1s, R1_PIECES[0:1],
                                          lambda p: ones256_sb[:, :, :]))

                rest = R1_PIECES[1:]
                for k in range(0, len(rest), 2):
                    chunk = rest[k:k + 2]
                    qs.append(lambda ch=chunk: pair_mm(
                        s, r1s, ch, lambda p: ones256_sb[:, :, :]))

                def q_exact_head():
                    rec1x = xsp.tile([128, 128], F32, tag="rec1x")
                    nc.vector.reciprocal_approx_fast(out=rec1x, in_=r1s[:, 0:128])
                    p1m = xsp.tile([128, 128], F32, tag="p1m")
                    nc.vector.scalar_tensor_tensor(
                        out=p1m, in0=s["e1"][:, 0:128],
                        scalar=cml256_sb[:, bi:bi + 1], in1=rec1x,
                        op0=ALU.mult, op1=ALU.mult,
                    )
                    s["p1m"] = p1m

                qs.append(q_exact_head)

                for k in range(0, len(PV_PIECES), 2):
                    chunk = PV_PIECES[k:k + 2]
                    qs.append(lambda ch=chunk: pair_mm(
                        s, otps, ch, lambda p: s["vcp"][:, p, :, :]))

                # exact-path matmul of the PREVIOUS block (its e2x landed at
                # the end of the previous ACT slot); doubles as PE spacing
                # between the last PV piece and the otps reads below
                if blk >= 1:
                    qs.append(lambda: exact_mm(blk - 1))

                def q_grec():
                    if DEBUG_DUMP and blk == DBG_BLK:
                        r1c = finp.tile([128, 1024], F32, tag="r1c")
                        nc.vector.tensor_copy(out=r1c, in_=r1s[:, 0:1024])
                        nc.sync.dma_start(out=r1dump_d[:, :], in_=r1c)
                        nc.sync.dma_start(out=e1dump_d[:, :], in_=s["e1"][:, :])
                    grec = finp.tile([128, 896], F32, tag="grec")
                    nc.vector.reciprocal_approx_fast(out=grec, in_=r1s[:, 128:1024])
                    s["grec"] = grec

                qs.append(q_grec)

                def q_fin_main():
                    if DEBUG_DUMP and blk == DBG_BLK:
                        otc = finp.tile([128, 1024], F32, tag="otc")
                        nc.vector.tensor_copy(out=otc, in_=otps[:, 0:1024])
                        nc.sync.dma_start(out=otdump_d[:, :], in_=otc)
                    t_sb = finp.tile([128, 896], BF16, tag="t")
                    nc.vector.tensor_mul(t_sb, otps[:, 128:1024], s["grec"])
                    out_sb = outp.tile([128, 1024], BF16, tag="out")
                    s["out"] = out_sb
                    nc.vector.tensor_scalar_add(
                        out_sb[:, 128:1024], t_sb, vt1024_sb[:, blk:blk + 1]
                    )

                qs.append(q_fin_main)

                # exact finalize of the previous block: the DVE is several
                # ops past PVex by now
                if blk >= 1:
                    qs.append(lambda: exact_fin(blk - 1))
                return qs

            def emit_e2x(blk):
                s = st[blk]
                e2x = xsp.tile([128, 128], BF16, tag="e2x")
                nc.scalar.activation(e2x, s["p1m"], AF.Exp)
                s["e2x"] = e2x

            def exact_mm(blk):
                s = st[blk]
                nc.tensor.matmul(
                    otps[:, 0:128], lhsT=s["vcm0"], rhs=s["e2x"],
                    start=True, stop=True, skip_group_check=True,
                )

            def exact_fin(blk):
                bi, h = divmod(blk, H)
                s = st[blk]
                nc.vector.tensor_scalar(
                    out=s["out"][:, 0:128], in0=otps[:, 0:128],
                    scalar1=vt0adj_sb[:, blk:blk + 1], scalar2=1.0 / 256.0,
                    op0=ALU.add, op1=ALU.mult,
                )
                nc.sync.dma_start(
                    out=out1_d[bi, DK * h:DK * (h + 1), :], in_=s["out"][0:DK, :])
                nc.sync.dma_start(
                    out=out2_d[bi, DK * h:DK * (h + 1), :], in_=s["out"][DK:2 * DK, :])

            # ---- main pipeline ----
            dma_in(0)
            for i in range(NB + 1):
                dq = quanta(i - 1) if i >= 1 else []
                di = 0

                def drain(k):
                    nonlocal di
                    n = min(k, len(dq) - di)
                    for _ in range(n):
                        dq[di]()
                        di += 1

                if i < NB:
                    if i + 1 < NB:
                        dma_in(i + 1)
                    alloc_e1(i)
                    # exp1 lags scores by one window: the psum write pipeline
                    # of window w's matmuls drains while window w+1 is issued
                    for w in range(NW):
                        emit_scores(i, w)
                        if w >= 1:
                            emit_exp1(i, w - 1)
                        drain(4)
                    emit_exp1(i, NW - 1)
                    drain(len(dq))
                    if i >= 1:
                        emit_e2x(i - 1)
                else:
                    drain(len(dq))
                    emit_e2x(i - 1)
                    exact_mm(i - 1)
                    exact_fin(i - 1)

    nc.compile()
    return nc


_NC_CACHE = None


def _get_nc():
    global _NC_CACHE
    if _NC_CACHE is None:
        _NC_CACHE = build_nc()
    return _NC_CACHE


def make_in_maps(q, k, v1, v2, cm):
    """Full inputs -> per-core input maps (host-side sharding + layout)."""
    q = np.asarray(q, dtype=np.float32)
    k = np.asarray(k, dtype=np.float32)
    v1 = np.asarray(v1, dtype=np.float32)
    v2 = np.asarray(v2, dtype=np.float32)
    cm = np.asarray(cm)

    id8 = np.eye(128, dtype=NPF8)
    # additive causal mask on the diag block: -448 where k >= q else 0
    dm8 = np.where(
        np.arange(128)[:, None] >= np.arange(128)[None, :], -240.0, 0.0
    ).astype(NPF8)
    ones256 = np.full((128, 2, 128), 128.0, NPF8)

    in_maps = []
    for core in range(NCORES):
        b0 = core * BLOC
        qt = np.ascontiguousarray(
            q[b0:b0 + BLOC].reshape(BLOC, S, H, DK).transpose(0, 2, 3, 1)
        ).astype(NPF8)
        kt = np.ascontiguousarray(
            k[b0:b0 + BLOC].reshape(BLOC, S, H, DK).transpose(0, 2, 3, 1)
        ).astype(NPF8)
        cml = 1.0 - cm[b0:b0 + BLOC].astype(np.float32)      # [BLOC, S]
        # v concat: [BLOC, H, key(S), d(128)]
        v1s = v1[b0:b0 + BLOC].reshape(BLOC, S, H, DK).transpose(0, 2, 1, 3)
        v2s = v2[b0:b0 + BLOC].reshape(BLOC, S, H, DK).transpose(0, 2, 1, 3)
        vc = np.concatenate([v1s, v2s], axis=3)               # [BLOC,H,S,128]
        vtot = vc.astype(np.float64).sum(axis=2).astype(np.float32)  # [BLOC,H,128]
        vcm8 = vc * (cml[:, None, :, None] * 0.125)           # masked /8
        # pair weights: [BLOC, H, pair, key128, slot, d]
        vcp = np.ascontiguousarray(
            vcm8.reshape(BLOC, H, NCH, 128, 128)
                .reshape(BLOC, H, 4, 2, 128, 128)
                .transpose(0, 1, 4, 2, 3, 5)
        ).astype(NPF8)
        vcm0 = np.ascontiguousarray(2.0 * vcm8[:, :, 0:128, :])
        vcm0 = vcm0.astype(NPBF16)                            # [BLOC,H,128,128]
        vt1024 = np.ascontiguousarray(
            (vtot / 1024.0).reshape(NB, 128).T.astype(np.float32))
        vt0adj = np.ascontiguousarray(
            ((vtot - (vc[:, :, 0:128, :] * cml[:, None, 0:128, None]).sum(2))
             / 4.0).reshape(NB, 128).T.astype(np.float32))
        cml256 = np.ascontiguousarray((128.0 * cml[:, 0:128]).T.astype(np.float32))
        in_maps.append(
            dict(qt=qt, kt=kt, vcp=vcp, vcm0=vcm0, vt1024=vt1024,
                 vt0adj=vt0adj, cml256=cml256, ones256=ones256,
                 id8=id8, dm8=dm8)
        )
    return in_maps


def _gather(res):
    out1 = np.concatenate(
        [np.asarray(r["out1t"]).astype(np.float32).transpose(0, 2, 1)
         for r in res.results], axis=0)
    out2 = np.concatenate(
        [np.asarray(r["out2t"]).astype(np.float32).transpose(0, 2, 1)
         for r in res.results], axis=0)
    out1[:, 0, :] = 0.0
    out2[:, 0, :] = 0.0
    return np.ascontiguousarray(out1), np.ascontiguousarray(out2)


def kernel(q, k, v1, v2, counter_attention_mask):
    global LAST_RESULTS
    in_maps = make_in_maps(q, k, v1, v2, counter_attention_mask)
    nc = _get_nc()
    res = run_bass_kernel_spmd(
        nc, in_maps, core_ids=list(range(NCORES)), trace=TRACE
    )
    LAST_RESULTS = res
    return _gather(res)


# revision 17
# speedup vs baseline: 1.0842x; 1.0078x over previous
"""Trainium2 Bass kernel for dual-attention (DisKT-style) nn module.

Math per (batch, head), S=1024, dk=64, [k, q] layout on-chip:
    sT   = (k_h @ q_h^T) + causal(-448 fp8 fixup)       fp8 matmuls
    e1   = fp8(exp(sT/8))                               ACT, fp8 out
    r1s  = 256 * sum_k e1[k, q]                         fp8 pair-DoubleRow
    otps = (vcm/4)^T @ e1                               fp8 pair-DoubleRow
    out[q>=128] = otps * recip(r1s) + vtot/1024
    exact island q<128 (second-softmax exp matters there):
      p1m  = e1 * (256*cml[k]) * recip(r1s[0:128])
      e2x  = exp(p1m)   (==1 at masked/dead keys)
      out[0:128] = ((vcm0/4)^T @ e2x + vtot0adj/4) / 256

Key simplifications vs a literal translation of the reference:
  - second softmax denominator = 1024 + sum cml*(exp(p1)-1) is in
    [1024, 1025] since sum p1 <= 1: approximated by 1024 (<=1e-3 rel).
  - the "+1 per key" of the second softmax contributes vtot[d]*rec2 ~=
    vtot/1024: folded into a per-partition scalar add (exact vtot).
  - exp(p1) ~= 1 + p1 for q >= 128 (p1 <= ~0.1): the PV/r1 matmuls run
    directly on e1 and the 1/r1 scale folds into the output pass.
  - chunk-pair DoubleRow fp8: moving AP [128, (delta,2), (1,W)] feeds
    two 128-key chunks per pass instruction at 0.5 cyc/col.

Sharding: data-parallel over batch, B=16 -> 2 per core on 8 cores.
"""

import numpy as np
import ml_dtypes

import concourse.bass as bass
import concourse.mybir as mybir
import concourse.tile as tile
from concourse import bacc
from concourse.ap import AP
from concourse.bass_utils import run_bass_kernel_spmd

B, S, D, H = 16, 1024, 512, 8
DK = D // H           # 64
NCORES = 8
BLOC = B // NCORES    # 2 batches per core
NB = BLOC * H         # 16 blocks per core
NCH = S // 128        # 8 k-chunks of 128
F32 = mybir.dt.float32
BF16 = mybir.dt.bfloat16
F8 = mybir.dt.float8e4
DR = mybir.MatmulPerfMode.DoubleRow
NPBF16 = ml_dtypes.bfloat16
NPF8 = ml_dtypes.float8_e4m3

LIVE = [S - 128 * c for c in range(NCH)]          # live width per chunk
OFF = [sum(LIVE[:c]) for c in range(NCH)]         # packed offset per chunk
PACK = OFF[-1] + LIVE[-1]                         # 4608
E1W = PACK + 128                                  # +128 zero cols for A-only
NW = (PACK + 1023) // 1024                        # 5 exp windows
WLEN = [min(1024, PACK - 1024 * w) for w in range(NW)]

# knobs test.py can flip
TRACE = False
DEBUG_DUMP = False
DBG_BLK = 0
LAST_RESULTS = None


def _scores_segs():
    """[(w, s0, s1, c, d0, d1)]: packed-col segments per exp window, split
    at 512 psum-bank boundaries; (d0, d1) = local diag-fixup range or None"""
    segs = []
    for c in range(NCH):
        g0, g1 = OFF[c], OFF[c] + LIVE[c]
        bounds = sorted({g0, g1} | {x for x in range(0, PACK + 512, 512)
                                    if g0 < x < g1})
        for s0, s1 in zip(bounds[:-1], bounds[1:]):
            w = s0 // 1024
            dm = None
            if s0 < g0 + 128:
                dm = (s0 - g0, min(s1 - g0, 128))
            segs.append((w, s0, s1, c, dm))
    return segs


SCORE_SEGS = _scores_segs()


def _pair_pieces(include_cell0):
    """[(pair, a, b, start, stop, aonly)] for a pair-DoubleRow sweep over e1.
    Cells of 256 q-cols; pair p = chunks (2p, 2p+1) covers q >= 256p; the
    first 128 cols of a pair's own cell are A-only (B reads the zero pad).
    CELL-major: each cell's accumulation group opens and closes before the
    next cell starts -- the psum hardware allows only ONE open group per
    bank, so groups must never interleave within a bank."""
    out = []
    for cell in range(4):
        mem = []
        for p in range(cell):
            mem.append([p, 256 * cell, 256 * cell + 256, False, False, False])
        if include_cell0 or cell > 0:
            mem.append([cell, 256 * cell, 256 * cell + 128, False, True, True])
        mem.append([cell, 256 * cell + 128, 256 * cell + 256, False, True, False])
        if cell == 0:
            for m in mem:
                m[3] = True
        else:
            mem[0][3] = True
        out += [(m[0], m[1], m[2], m[3], m[4], m[5]) for m in mem]
    return out


R1_PIECES = _pair_pieces(True)     # 15 pieces, covers q in [0, 1024)
PV_PIECES = _pair_pieces(False)    # 14 pieces, covers q in [128, 1024)


def build_nc(debug=False):
    nc = bacc.Bacc("TRN2", target_bir_lowering=False, debug=debug)
    AF = mybir.ActivationFunctionType
    ALU = mybir.AluOpType

    qt_d = nc.dram_tensor("qt", [BLOC, H, DK, S], F8, kind="ExternalInput")
    kt_d = nc.dram_tensor("kt", [BLOC, H, DK, S], F8, kind="ExternalInput")
    # pair-DR PV weights: vcm/4 as [pair, key, slot, d]
    vcp_d = nc.dram_tensor("vcp", [BLOC, H, 128, 4, 2, 128], F8,
                           kind="ExternalInput")
    # exact-path chunk-0 weights (cml*v)/4
    vcm0_d = nc.dram_tensor("vcm0", [BLOC, H, 128, 128], BF16,
                            kind="ExternalInput")
    vt1024_d = nc.dram_tensor("vt1024", [128, NB], F32, kind="ExternalInput")
    vt0adj_d = nc.dram_tensor("vt0adj", [128, NB], F32, kind="ExternalInput")
    cml256_d = nc.dram_tensor("cml256", [128, BLOC], F32, kind="ExternalInput")
    ones256_d = nc.dram_tensor("ones256", [128, 2, 128], F8, kind="ExternalInput")
    id8_d = nc.dram_tensor("id8", [128, 128], F8, kind="ExternalInput")
    dm8_d = nc.dram_tensor("dm8", [128, 128], F8, kind="ExternalInput")
    out1_d = nc.dram_tensor("out1t", [BLOC, D, S], BF16, kind="ExternalOutput")
    out2_d = nc.dram_tensor("out2t", [BLOC, D, S], BF16, kind="ExternalOutput")
    if DEBUG_DUMP:
        e1dump_d = nc.dram_tensor("e1dump", [128, E1W], F8, kind="ExternalOutput")
        r1dump_d = nc.dram_tensor("r1dump", [128, 1024], F32, kind="ExternalOutput")
        otdump_d = nc.dram_tensor("otdump", [128, 1024], F32, kind="ExternalOutput")

    with tile.TileContext(nc) as tc:
        with (
            tc.tile_pool(name="consts", bufs=1) as consts,
            tc.tile_pool(name="vc", bufs=3) as vcp_p,
            tc.tile_pool(name="xs", bufs=2) as xsp,
            tc.tile_pool(name="fin", bufs=2) as finp,
            tc.tile_pool(name="outs", bufs=2) as outp,
            tc.tile_pool(name="ps", bufs=1, space="PSUM") as psp,
        ):
            id8_sb = consts.tile([128, 128], F8)
            nc.sync.dma_start(out=id8_sb, in_=id8_d[:, :])
            dm8_sb = consts.tile([128, 128], F8)
            nc.sync.dma_start(out=dm8_sb, in_=dm8_d[:, :])
            ones256_sb = consts.tile([128, 2, 128], F8)
            nc.sync.dma_start(out=ones256_sb, in_=ones256_d[:])
            vt1024_sb = consts.tile([128, NB], F32)
            nc.sync.dma_start(out=vt1024_sb, in_=vt1024_d[:, :])
            vt0adj_sb = consts.tile([128, NB], F32)
            nc.sync.dma_start(out=vt0adj_sb, in_=vt0adj_d[:, :])
            cml256_sb = consts.tile([128, BLOC], F32)
            nc.sync.dma_start(out=cml256_sb, in_=cml256_d[:, :])
            negtwo_sb = consts.tile([128, 1], F32, name="negtwo")
            nc.vector.memset(negtwo_sb, -3.5)

            # persistent 3-deep rings for qt/kt/e1 (stable identity so the
            # one-time pad memsets cover all blocks)
            qt_t = [consts.tile([128, S], F8, name=f"qtr{r}") for r in range(4)]
            kt_t = [consts.tile([128, S], F8, name=f"ktr{r}") for r in range(4)]
            e1_t = [consts.tile([128, E1W], F8, name=f"e1r{r}") for r in range(3)]
            for r in range(4):
                nc.vector.memset(qt_t[r][DK:128, :], 0.0)
                nc.vector.memset(kt_t[r][DK:128, :], 0.0)
            for r in range(3):
                nc.vector.memset(e1_t[r][:, PACK:E1W], 0.0)

            # persistent psum: scores ring (2x 1024), otps, r1s
            ring = psp.tile([128, 2048], F32, name="ring")
            otps = psp.tile([128, 1024], F32, name="otps")
            r1s = psp.tile([128, 1024], F32, name="r1s")

            st = [dict() for _ in range(NB)]

            # PE warmup (DVFS pre-ramp) on a zeroed scratch
            wu_sb = consts.tile([128, 512], BF16, name="wu_sb")
            nc.vector.memset(wu_sb, 0.0)
            for r in range(12):
                nc.tensor.matmul(
                    ring[:, 0:512], lhsT=wu_sb[:, 0:128], rhs=wu_sb,
                    start=True, stop=True, skip_group_check=True,
                )

            def dma_in(blk):
                bi, h = divmod(blk, H)
                s = st[blk]
                qt_sb = qt_t[blk % 4]
                kt_sb = kt_t[blk % 4]
                nc.sync.dma_start(out=qt_sb[0:DK, :], in_=qt_d[bi, h])
                nc.sync.dma_start(out=kt_sb[0:DK, :], in_=kt_d[bi, h])
                vcp_sb = vcp_p.tile([128, 4, 2, 128], F8, tag="vcp")
                nc.sync.dma_start(out=vcp_sb, in_=vcp_d[bi, h])
                vcm0_sb = vcp_p.tile([128, 128], BF16, tag="vcm0")
                nc.sync.dma_start(out=vcm0_sb, in_=vcm0_d[bi, h])
                s["qt"], s["kt"], s["vcp"], s["vcm0"] = qt_sb, kt_sb, vcp_sb, vcm0_sb

            def alloc_e1(blk):
                st[blk]["e1"] = e1_t[blk % 3]

            def emit_scores(blk, w):
                s = st[blk]
                base = (w % 2) * 1024
                g0 = 1024 * w
                for (sw, s0, s1, c, dm) in SCORE_SEGS:
                    if sw != w:
                        continue
                    loc = base + s0 - g0
                    qa = s0 - OFF[c] + 128 * c
                    nc.tensor.matmul(
                        ring[:, loc:loc + (s1 - s0)],
                        lhsT=s["kt"][:, 128 * c:128 * c + 128],
                        rhs=s["qt"][:, qa:qa + (s1 - s0)],
                        start=True, stop=dm is None, skip_group_check=True,
                    )
                    if dm is not None:
                        d0, d1 = dm
                        nc.tensor.matmul(
                            ring[:, loc:loc + (d1 - d0)],
                            lhsT=id8_sb, rhs=dm8_sb[:, d0:d1],
                            start=False, stop=True, skip_group_check=True,
                        )

            def emit_exp1(blk, w):
                s = st[blk]
                base = (w % 2) * 1024
                # bias -3.5: e1 scaled by e^-3.5 so fp8 max 240 is never hit
                # (global max score ~67.8 -> e1max ~144)
                # (every consumer is scale-invariant in e1)
                nc.scalar.activation(
                    s["e1"][:, 1024 * w:1024 * w + WLEN[w]],
                    ring[:, base:base + WLEN[w]],
                    AF.Exp, scale=0.125, bias=negtwo_sb[:, 0:1],
                )

            def pair_mm(s, psum, pieces_sel, lhsT_of):
                """emit pair-DR matmuls for the given piece list"""
                e1t = s["e1"]
                for (p, a, b, st_, sp_, aonly) in pieces_sel:
                    a0 = OFF[2 * p] + (a - 256 * p)
                    delta = (PACK - a0) if aonly else (LIVE[2 * p] - 128)
                    rhs = AP(e1t[:, 0:1].tensor, a0,
                             [[E1W, 128], [delta, 2], [1, b - a]])
                    nc.tensor.matmul(
                        psum[:, a:b], lhsT=lhsT_of(p), rhs=rhs,
                        start=st_, stop=sp_, perf_mode=DR,
                        skip_group_check=True,
                    )

            def quanta(blk):
                """PE/DVE/DMA quanta for block blk's passes + finalize, plus
                the exact-path tail of block blk-1.  Ordering rule: a psum
                READ (DVE) is never emitted right after its producing matmul
                -- several quanta of unrelated PE work sit in between so the
                PE psum-write pipeline has drained by the time the read's
                semaphore fires (observed transient-garbage reads on HW
                otherwise)."""
                bi, h = divmod(blk, H)
                s = st[blk]
                qs = []

                # r1 pass piece for cell 0 first (unlocks exact path)
                qs.append(lambda: pair_mm(s, r1s, R1_PIECES[0:1],
                                          lambda p: ones256_sb[:, :, :]))

                rest = R1_PIECES[1:]
                for k in range(0, len(rest), 2):
                    chunk = rest[k:k + 2]
                    qs.append(lambda ch=chunk: pair_mm(
                        s, r1s, ch, lambda p: ones256_sb[:, :, :]))

                def q_exact_head():
                    rec1x = xsp.tile([128, 128], F32, tag="rec1x")
                    nc.vector.reciprocal_approx_fast(out=rec1x, in_=r1s[:, 0:128])
                    p1m = xsp.tile([128, 128], F32, tag="p1m")
                    nc.vector.scalar_tensor_tensor(
                        out=p1m, in0=s["e1"][:, 0:128],
                        scalar=cml256_sb[:, bi:bi + 1], in1=rec1x,
                        op0=ALU.mult, op1=ALU.mult,
                    )
                    s["p1m"] = p1m

                qs.append(q_exact_head)

                for k in range(0, len(PV_PIECES), 2):
                    chunk = PV_PIECES[k:k + 2]
                    qs.append(lambda ch=chunk: pair_mm(
                        s, otps, ch, lambda p: s["vcp"][:, p, :, :]))

                # exact-path matmul of the PREVIOUS block (its e2x landed at
                # the end of the previous ACT slot); doubles as PE spacing
                # between the last PV piece and the otps reads below
                if blk >= 1:
                    qs.append(lambda: exact_mm(blk - 1))

                def q_grec():
                    if DEBUG_DUMP and blk == DBG_BLK:
                        r1c = finp.tile([128, 1024], F32, tag="r1c")
                        nc.vector.tensor_copy(out=r1c, in_=r1s[:, 0:1024])
                        nc.sync.dma_start(out=r1dump_d[:, :], in_=r1c)
                        nc.sync.dma_start(out=e1dump_d[:, :], in_=s["e1"][:, :])
                    grec = finp.tile([128, 896], F32, tag="grec")
                    nc.vector.reciprocal_approx_fast(out=grec, in_=r1s[:, 128:1024])
                    s["grec"] = grec

                qs.append(q_grec)

                def q_fin_main():
                    if DEBUG_DUMP and blk == DBG_BLK:
                        otc = finp.tile([128, 1024], F32, tag="otc")
                        nc.vector.tensor_copy(out=otc, in_=otps[:, 0:1024])
                        nc.sync.dma_start(out=otdump_d[:, :], in_=otc)
                    t_sb = finp.tile([128, 896], BF16, tag="t")
                    nc.vector.tensor_mul(t_sb, otps[:, 128:1024], s["grec"])
                    out_sb = outp.tile([128, 1024], BF16, tag="out")
                    s["out"] = out_sb
                    nc.vector.tensor_scalar_add(
                        out_sb[:, 128:1024], t_sb, vt1024_sb[:, blk:blk + 1]
                    )

                qs.append(q_fin_main)

                # exact finalize of the previous block: the DVE is several
                # ops past PVex by now
                if blk >= 1:
                    qs.append(lambda: exact_fin(blk - 1))
                return qs

            def emit_e2x(blk):
                s = st[blk]
                e2x = xsp.tile([128, 128], BF16, tag="e2x")
                nc.scalar.activation(e2x, s["p1m"], AF.Exp)
                s["e2x"] = e2x

            def exact_mm(blk):
                s = st[blk]
                nc.tensor.matmul(
                    otps[:, 0:128], lhsT=s["vcm0"], rhs=s["e2x"],
                    start=True, stop=True, skip_group_check=True,
                )

            def exact_fin(blk):
                bi, h = divmod(blk, H)
                s = st[blk]
                nc.vector.tensor_scalar(
                    out=s["out"][:, 0:128], in0=otps[:, 0:128],
                    scalar1=vt0adj_sb[:, blk:blk + 1], scalar2=1.0 / 256.0,
                    op0=ALU.add, op1=ALU.mult,
                )
                nc.sync.dma_start(
                    out=out1_d[bi, DK * h:DK * (h + 1), :], in_=s["out"][0:DK, :])
                nc.sync.dma_start(
                    out=out2_d[bi, DK * h:DK * (h + 1), :], in_=s["out"][DK:2 * DK, :])

            # ---- main pipeline ----
            dma_in(0)
            dma_in(1)
            for i in range(NB + 1):
                dq = quanta(i - 1) if i >= 1 else []
                di = 0

                def drain(k):
                    nonlocal di
                    n = min(k, len(dq) - di)
                    for _ in range(n):
                        dq[di]()
                        di += 1

                if i < NB:
                    if i + 2 < NB:
                        dma_in(i + 2)
                    alloc_e1(i)
                    # exp1 lags scores by one window: the psum write pipeline
                    # of window w's matmuls drains while window w+1 is issued
                    for w in range(NW):
                        emit_scores(i, w)
                        if w >= 1:
                            emit_exp1(i, w - 1)
                        drain(4)
                    emit_exp1(i, NW - 1)
                    drain(len(dq))
                    if i >= 1:
                        emit_e2x(i - 1)
                else:
                    drain(len(dq))
                    emit_e2x(i - 1)
                    exact_mm(i - 1)
                    exact_fin(i - 1)

    nc.compile()
    return nc


_NC_CACHE = None


def _get_nc():
    global _NC_CACHE
    if _NC_CACHE is None:
        _NC_CACHE = build_nc()
    return _NC_CACHE


def make_in_maps(q, k, v1, v2, cm):
    """Full inputs -> per-core input maps (host-side sharding + layout)."""
    q = np.asarray(q, dtype=np.float32)
    k = np.asarray(k, dtype=np.float32)
    v1 = np.asarray(v1, dtype=np.float32)
    v2 = np.asarray(v2, dtype=np.float32)
    cm = np.asarray(cm)

    id8 = np.eye(128, dtype=NPF8)
    # additive causal mask on the diag block: -448 where k >= q else 0
    dm8 = np.where(
        np.arange(128)[:, None] >= np.arange(128)[None, :], -240.0, 0.0
    ).astype(NPF8)
    ones256 = np.full((128, 2, 128), 128.0, NPF8)

    in_maps = []
    for core in range(NCORES):
        b0 = core * BLOC
        qt = np.ascontiguousarray(
            q[b0:b0 + BLOC].reshape(BLOC, S, H, DK).transpose(0, 2, 3, 1)
        ).astype(NPF8)
        kt = np.ascontiguousarray(
            k[b0:b0 + BLOC].reshape(BLOC, S, H, DK).transpose(0, 2, 3, 1)
        ).astype(NPF8)
        cml = 1.0 - cm[b0:b0 + BLOC].astype(np.float32)      # [BLOC, S]
        # v concat: [BLOC, H, key(S), d(128)]
        v1s = v1[b0:b0 + BLOC].reshape(BLOC, S, H, DK).transpose(0, 2, 1, 3)
        v2s = v2[b0:b0 + BLOC].reshape(BLOC, S, H, DK).transpose(0, 2, 1, 3)
        vc = np.concatenate([v1s, v2s], axis=3)               # [BLOC,H,S,128]
        vtot = vc.astype(np.float64).sum(axis=2).astype(np.float32)  # [BLOC,H,128]
        vcm8 = vc * (cml[:, None, :, None] * 0.125)           # masked /8
        # pair weights: [BLOC, H, pair, key128, slot, d]
        vcp = np.ascontiguousarray(
            vcm8.reshape(BLOC, H, NCH, 128, 128)
                .reshape(BLOC, H, 4, 2, 128, 128)
                .transpose(0, 1, 4, 2, 3, 5)
        ).astype(NPF8)
        vcm0 = np.ascontiguousarray(2.0 * vcm8[:, :, 0:128, :])
        vcm0 = vcm0.astype(NPBF16)                            # [BLOC,H,128,128]
        vt1024 = np.ascontiguousarray(
            (vtot / 1024.0).reshape(NB, 128).T.astype(np.float32))
        vt0adj = np.ascontiguousarray(
            ((vtot - (vc[:, :, 0:128, :] * cml[:, None, 0:128, None]).sum(2))
             / 4.0).reshape(NB, 128).T.astype(np.float32))
        cml256 = np.ascontiguousarray((128.0 * cml[:, 0:128]).T.astype(np.float32))
        in_maps.append(
            dict(qt=qt, kt=kt, vcp=vcp, vcm0=vcm0, vt1024=vt1024,
                 vt0adj=vt0adj, cml256=cml256, ones256=ones256,
                 id8=id8, dm8=dm8)
        )
    return in_maps


def _gather(res):
    out1 = np.concatenate(
        [np.asarray(r["out1t"]).astype(np.float32).transpose(0, 2, 1)
         for r in res.results], axis=0)
    out2 = np.concatenate(
        [np.asarray(r["out2t"]).astype(np.float32).transpose(0, 2, 1)
         for r in res.results], axis=0)
    out1[:, 0, :] = 0.0
    out2[:, 0, :] = 0.0
    return np.ascontiguousarray(out1), np.ascontiguousarray(out2)


def kernel(q, k, v1, v2, counter_attention_mask):
    global LAST_RESULTS
    in_maps = make_in_maps(q, k, v1, v2, counter_attention_mask)
    nc = _get_nc()
    res = run_bass_kernel_spmd(
        nc, in_maps, core_ids=list(range(NCORES)), trace=TRACE
    )
    LAST_RESULTS = res
    return _gather(res)
